# revision 15
# baseline (speedup 1.0000x reference)
"""Draft (block-sparse) attention kernel for Trainium2, 8 NeuronCores.

Strategy (v2)
-------------
* Head-parallel sharding: 16 heads -> 8 cores x 2 heads (361 kept blocks
  per head -> 722 (qb, kb) block pairs per core).
* Inspector / executor split: the draft map (pooled 60x60 attention +
  top-10% percentile mask) is a bitwise replica of the reference's jax
  ops on XLA-CPU; the block schedule is baked into the Bass program.
* Executor dataflow per pair:
      S'[kb_tok, qb_tok] = (K_kb)(Q'_qb)^T     PE fp16, K=64 contraction
                                               (no zero padding needed -
                                               matmul cost is column
                                               count, independent of K)
      P = 2^(S')                               Q' is pre-scaled on host
                                               by log2(e)/8, so softmax's
                                               exp(s/8) becomes 2^s'.
      acc[qb] += P^T @ [V_kb | 1]              PE fp16 -> PSUM fp32; the
                                               ones column accumulates
                                               the softmax denominator.
  The PSUM accumulators ship to HBM as fp16 (numerator + denominator);
  the host does the final divide, restore permutation and zero rows.
* exp is the serial bottleneck on one engine (ScalarE ~86us wall-to-wall
  in v1), so it is SPLIT between ScalarE (ACT Exp, exact) and VectorE
  (DVE): chunks of 12 pairs alternate between the engines by a weighted
  round-robin.  GPSIMD/Pool cannot read PSUM (BIR verifier) so it only
  issues DMA and memsets.
* DVE exp method (DVE_EXP):
    'pow'   - tensor_tensor(out, base=2.0 tile, s', AluOpType.pow)
    'trick' - int16 Schraudolph: y=s'*1024+B truncated to int16 IS the
              fp16 bit pattern of ~2^s' (max rel err ~3%).  To protect
              accuracy, chunks are weight-routed: within each 6-query-row
              window, pairs are sorted by draft-map weight relative to
              their row's max, ACT (exact) takes the heavy pairs, DVE the
              light tail.
* Output: per 6-row window a PSUM tile [128, 6*65] accumulates; after the
  window's last PV matmul it is copied (ACT/DVE alternating) to an SBUF
  staging tile as fp16 and DMA'd out immediately - no end-of-kernel DMA
  bulge.
* One tc.Switch (computed goto) dispatches the 8 per-core bodies instead
  of a binary If-tree (up to 3 far jumps -> 1).
"""

import math

import numpy as np

# ---------------------------------------------------------------- constants
L = 7680          # visual tokens (2 frames x 48 x 80)
NH = 16           # heads
D = 64            # head dim
S = 60            # pooled tokens = sparse blocks per side
BLK = 128         # tokens per block (L // S)
NCORES = 8
HPC = NH // NCORES  # heads per core
POOL_H, POOL_W, LATENT_H, LATENT_W = 8, 16, 48, 80
SPARSITY = 0.9

CHUNK_A = 12      # pairs per ScalarE exp chunk (rate-matched: 12*0.833ns)
CHUNK_V = 8       # pairs per DVE exp chunk (8*1.042ns); A/V strictly alternate
CHUNK = CHUNK_A   # PSUM chunk tile size [128, CHUNK*128] = 3 banks
PVPACK = 6        # query rows per PSUM accumulator tile [128, 6*65]
NWIN = S // PVPACK  # windows (pv tiles) per head
MMDT = np.float16

FA = 0.55         # fraction of exp chunks on ScalarE (ACT); rest on DVE
DVE_EXP = "expbits"  # 'expbits' (custom DVE op, ~2e-3) or 'trick' (~3e-2)

# Scores are shipped so the PE emits y'' = 1024*log2(p_target) - 512:
# q is pre-scaled by 128*log2(e) (so s_raw*that = 1024*log2(e)/8*s_raw)
# and a 65th constant contraction row (q=-512, k=1) adds the -512.
QSCALE = 128.0 * math.log2(math.e)
CROW_Q = -512.0
CROW_K = 1.0

# EXPBITS custom-DVE op: magic-add floor-reduction + quadratic mantissa
# correction, B folded into the poly constant; the int16 output conversion
# + fp16 bitcast assemble the final float.  Max ripple ~2.1e-3; the mean
# +8.75-bit offset is matched on the ACT path via its activation bias so
# the two engines' outputs mix without a systematic scale step.
M2 = 1.5 * 2**33
EXPB_C0B = 15793.353872778089
EXPB_C1 = 1.0001873677319373
EXPB_C2 = 3.30059028135e-4
EXPB_K = 0.005906502815287413          # ln of the DVE path's mean scale
ACT_SCALE = math.log(2.0) / 1024.0     # exp(y''*scale + bias) == p_target
ACT_BIAS = 512.0 * math.log(2.0) / 1024.0 + EXPB_K
TRICK_B = 15360.0 + 512.0 - 44.0       # fallback: bits = y'' + TRICK_B


def _expbits_ref(in0, in1, s0, s1, imm2):
    ypp = in0.astype(np.float32)
    u = (ypp + np.float32(M2)).astype(np.float32)
    iF = (u - np.float32(M2)).astype(np.float32)
    f = (ypp - iF).astype(np.float32)
    t = ((np.float32(imm2) * f + np.float32(s1)) * f).astype(np.float32)
    t = (t + in1[:, :1].astype(np.float32)).astype(np.float32)
    t = (t + iF).astype(np.float32)
    return np.round(t).astype(np.float32)   # HW rounds on fp32->int16


def _register_expbits_op():
    import concourse.dve_ops as dvo
    from concourse.dve_spec import (Spec, Src0, C0, C1, C2, C3,
                                    _spill_c3_to_src1, lower)
    from concourse.dve_uop import DveOpSpec

    name = "EXPBITS_ANT"
    if name in dvo._SUB_OPCODE_FOR_NAME:
        return next(o for o in dvo.OPS if o.name == name)
    u = Src0 + C0
    iF = u - C0
    f = Src0 - iF
    body = ((C2 * f + C1) * f + C3) + iF
    spec = Spec(body=_spill_c3_to_src1(body), reference=_expbits_ref)
    row = dvo._CUSTOM_DVE_ROW_BASE + len(dvo.OPS)
    shas = {}
    for ver in ("v3", "v4"):
        tmp = DveOpSpec(name=name, opcode=row, uops=lower(spec, ver=ver),
                        rd1_en=True)
        shas[ver] = tmp.sha(ver)
    op = dvo.DveOp(name, spec, subdim=False, uops_sha=shas)
    dvo.OPS.append(op)
    dvo.CUSTOM_DVE_SPECS[name] = spec
    dvo._SUB_OPCODE_FOR_NAME[name] = row
    return op


def _reorg_restore():
    part = LATENT_W * POOL_H
    blk = LATENT_W
    sub = POOL_W
    bpp = part // blk
    spb = blk // sub
    pat = np.arange(part).reshape(bpp, spb, sub).transpose(1, 0, 2).reshape(-1)
    nparts = L // part
    reorg = (np.arange(nparts)[:, None] * part + pat[None, :]).reshape(-1)
    restore = np.argsort(reorg)
    return reorg, restore


def _inspector(qn: np.ndarray, kn: np.ndarray):
    """Replicate the reference draft-map + percentile mask bit-exactly on
    XLA-CPU.  Returns (mask, attn) - attn drives the exact-vs-approx exp
    routing."""
    import jax
    import jax.numpy as jnp

    with jax.default_device(jax.devices("cpu")[0]):
        q = jnp.asarray(qn)
        k = jnp.asarray(kn)
        nf = L // (LATENT_H * LATENT_W)

        def pool(x):
            x = x.reshape(nf, LATENT_H // POOL_H, POOL_H,
                          LATENT_W // POOL_W, POOL_W, NH, D)
            return x.mean(axis=(2, 4)).reshape(-1, NH, D)

        qs, ks = pool(q), pool(k)
        scores = jnp.einsum('lhd,mhd->hlm', qs, ks) / math.sqrt(D)
        attn = jax.nn.softmax(scores, axis=-1)
        n = S * S
        kk = int(SPARSITY * n)
        thr = jnp.sort(attn.reshape(NH, n), axis=-1)[:, kk - 1]
        mask = attn >= thr[:, None, None]
        return np.asarray(mask), np.asarray(attn)


def _schedule(mask_h: np.ndarray, attn_h: np.ndarray):
    """Per head: windows of PVPACK query rows.  Each window's pairs are
    sorted descending by draft weight relative to their row's max kept
    weight, so exact-exp chunks grab the heavy pairs first."""
    windows = []     # [ [(qb, kb), ...] sorted desc by relative weight ]
    zero_rows = []
    for qb in range(S):
        if not mask_h[qb].any():
            zero_rows.append(qb)
    for ti in range(NWIN):
        pairs = []
        for qb in range(ti * PVPACK, (ti + 1) * PVPACK):
            kbs = np.nonzero(mask_h[qb])[0]
            if len(kbs) == 0:
                continue
            rmax = attn_h[qb, kbs].max()
            for kb in kbs:
                pairs.append((qb, int(kb), float(attn_h[qb, kb] / rmax)))
        pairs.sort(key=lambda p: -p[2])
        windows.append([(qb, kb) for qb, kb, _ in pairs])
    return windows, zero_rows


# ---------------------------------------------------------------- builder
def _emit_loads(nc, pools, dram):
    """Input loads, identical instructions on every core.  Ordered by first
    use: kT0/qT0 gate head-0 compute start (kT fully - every kb block is
    live immediately; qT in ascending query order), vaug0 next, then all
    of head 1."""
    import concourse.mybir as mybir

    f16 = mybir.dt.float16
    f32 = mybir.dt.float32
    qT_ap, kT_ap, vaug_ap = dram[0], dram[1], dram[2]

    qT = [pools["io"].tile([65, L], f16, tag=f"qT{h}", name=f"qT{h}")
          for h in range(HPC)]
    kT = [pools["io"].tile([65, L], f16, tag=f"kT{h}", name=f"kT{h}")
          for h in range(HPC)]
    vaug = [pools["io"].tile([128, S * 65], f16, tag=f"vaug{h}", name=f"vg{h}")
            for h in range(HPC)]
    c0b = pools["io"].tile([128, 1], f32, tag="c0b", name="c0b")
    nc.gpsimd.memset(c0b[:, :], EXPB_C0B)
    abias = pools["io"].tile([128, 1], f32, tag="abias", name="abias")
    nc.gpsimd.memset(abias[:, :], ACT_BIAS)

    third = L // 3
    q1 = 1280
    q2 = 4480
    vh = S * 65 // 2
    # sync: kT0 thirds a, then early qT0 cols, vaug0 half, kT1 half
    nc.sync.dma_start(kT[0][:, 0:third], kT_ap[0][:, 0:third])
    nc.sync.dma_start(qT[0][:, 0:q1], qT_ap[0][:, 0:q1])
    nc.sync.dma_start(vaug[0][:, 0:vh], vaug_ap[0][:, 0:vh])
    nc.sync.dma_start(kT[1][:, 0:L // 2], kT_ap[1][:, 0:L // 2])
    # scalar: one kT0 third only, then ScalarE heads to its Switch arm
    nc.scalar.dma_start(kT[0][:, 2 * third:L], kT_ap[0][:, 2 * third:L])
    # gpsimd: the rest
    nc.gpsimd.dma_start(kT[0][:, third:2 * third], kT_ap[0][:, third:2 * third])
    nc.gpsimd.dma_start(qT[0][:, q1:q2], qT_ap[0][:, q1:q2])
    nc.gpsimd.dma_start(vaug[0][:, vh:], vaug_ap[0][:, vh:])
    nc.gpsimd.dma_start(qT[0][:, q2:L], qT_ap[0][:, q2:L])
    nc.gpsimd.dma_start(kT[1][:, L // 2:], kT_ap[1][:, L // 2:])
    nc.gpsimd.dma_start(qT[1][:, :], qT_ap[1][:, :])
    nc.gpsimd.dma_start(vaug[1][:, 0:vh], vaug_ap[1][:, 0:vh])
    nc.gpsimd.dma_start(vaug[1][:, vh:], vaug_ap[1][:, vh:])
    return qT, kT, vaug, c0b, abias


def _emit_core_compute(nc, tc, pools, tiles, dram, core, scheds, expop):
    import concourse.mybir as mybir

    f32 = mybir.dt.float32
    f16 = mybir.dt.float16
    i16 = mybir.dt.int16
    qT, kT, vaug, c0b, abias = tiles
    out_ap = dram[3]

    turn = ["A"]      # strict A/V alternation, carried across windows
    win_i = 0
    for h in range(HPC):
        windows, _ = scheds[h]
        for ti in range(NWIN):
            wpairs = windows[ti]
            if not wpairs:
                win_i += 1
                continue
            n = len(wpairs)
            # partition the weight-sorted pairs: A chunks (exact exp, heavy
            # pairs from the front), V chunks (approx, light tail), strictly
            # alternating with sizes matched to engine rates
            engs, plists = [], []
            front, back = 0, n
            while front < back:
                if turn[0] == "A":
                    sz = min(CHUNK_A, back - front)
                    plists.append(wpairs[front:front + sz])
                    front += sz
                    engs.append("A")
                    turn[0] = "V"
                else:
                    sz = min(CHUNK_V, back - front)
                    plists.append(wpairs[back - sz:back])
                    back -= sz
                    engs.append("V")
                    turn[0] = "A"

            pv = pools["pv"].tile([128, PVPACK * 65], f32, tag="pv",
                                  name=f"pv{core}_{win_i}")
            # stage 1: all chunks' S matmuls + exp (A/V alternating)
            pcs = []
            for e, plist in zip(engs, plists):
                cw = len(plist)
                sc = pools["schunk"].tile([128, CHUNK * BLK], f32,
                                          tag="schunk",
                                          name=f"sc{core}_{win_i}_{len(pcs)}")
                for i, (qb, kb) in enumerate(plist):
                    nc.tensor.matmul(
                        sc[:, i * BLK:(i + 1) * BLK],
                        lhsT=kT[h][:, kb * BLK:(kb + 1) * BLK],
                        rhs=qT[h][:, qb * BLK:(qb + 1) * BLK],
                        start=True, stop=True,
                    )
                nn = cw * BLK
                pc = pools["pchunk"].tile([128, CHUNK * BLK], f16,
                                          tag="pchunk",
                                          name=f"pc{core}_{win_i}_{len(pcs)}")
                if e == "A":
                    nc.scalar.activation(
                        pc[:, :nn], sc[:, :nn],
                        mybir.ActivationFunctionType.Exp,
                        scale=ACT_SCALE, bias=abias[:, :],
                    )
                elif DVE_EXP == "expbits":
                    nc.vector._custom_dve(
                        expop, out=pc[:, :nn].bitcast(i16), in0=sc[:, :nn],
                        in1=c0b[:, :], s0=M2, s1=EXPB_C1, imm2=EXPB_C2,
                    )
                else:
                    nc.vector.tensor_scalar(
                        pc[:, :nn].bitcast(i16), sc[:, :nn],
                        1.0, TRICK_B,
                        mybir.AluOpType.mult, mybir.AluOpType.add,
                    )
                pcs.append(pc)

            # stage 2: PV accumulation, each row's matmuls CONSECUTIVE so
            # PSUM start/stop groups never interleave
            row_pairs = {}
            for ci, plist in enumerate(plists):
                for i, (qb, kb) in enumerate(plist):
                    row_pairs.setdefault(qb, []).append((ci, i, kb))
            for qb in sorted(row_pairs):
                rp = row_pairs[qb]
                r = qb - ti * PVPACK
                for j, (ci, i, kb) in enumerate(rp):
                    nc.tensor.matmul(
                        pv[:, r * 65:r * 65 + 65],
                        lhsT=pcs[ci][:, i * BLK:(i + 1) * BLK],
                        rhs=vaug[h][:, kb * 65:(kb + 1) * 65],
                        start=(j == 0), stop=(j == len(rp) - 1),
                        skip_group_check=True,
                    )

            st = pools["stage"].tile([128, PVPACK * 65], f16, tag="stage",
                                     name=f"st{core}_{win_i}")
            nc.vector.tensor_copy(st[:, :], pv[:, :])
            q = nc.sync if (win_i % 2 == 0) else nc.gpsimd
            q.dma_start(out_ap[h][ti], st[:, :])
            win_i += 1


def _build_program(scheds_by_core, expop):
    from contextlib import ExitStack

    import concourse.mybir as mybir
    import concourse.tile as tile
    from concourse import bacc

    f16 = mybir.dt.float16
    nc = bacc.Bacc("TRN2", target_bir_lowering=False, debug=False,
                   num_devices=NCORES)
    qT_ap = nc.dram_tensor("qT", [HPC, 65, L], f16, kind="ExternalInput").ap()
    kT_ap = nc.dram_tensor("kT", [HPC, 65, L], f16, kind="ExternalInput").ap()
    vaug_ap = nc.dram_tensor("vaug", [HPC, BLK, S * 65], f16,
                             kind="ExternalInput").ap()
    out_ap = nc.dram_tensor("out", [HPC, NWIN, BLK, PVPACK * 65], f16,
                            kind="ExternalOutput").ap()

    dram = (qT_ap, kT_ap, vaug_ap, out_ap)

    with tile.TileContext(nc) as tc:
        with ExitStack() as ctx:
            pools = {
                "io": ctx.enter_context(tc.tile_pool(name="io", bufs=1)),
                "schunk": ctx.enter_context(
                    tc.tile_pool(name="schunk", bufs=2, space="PSUM")),
                "pchunk": ctx.enter_context(
                    tc.tile_pool(name="pchunk", bufs=8)),
                "pv": ctx.enter_context(
                    tc.tile_pool(name="pv", bufs=2, space="PSUM")),
                "stage": ctx.enter_context(tc.tile_pool(name="stage", bufs=4)),
            }
            tiles = _emit_loads(nc, pools, dram)
            pid = nc.partition_id()

            def emit(core):
                _emit_core_compute(nc, tc, pools, tiles, dram, core,
                                   scheds_by_core[core], expop)

            with tc.If(pid < 4) as c1:
                with tc.If(pid < 2) as c2:
                    with tc.If(pid < 1) as c3:
                        emit(0)
                    with c3.Else():
                        emit(1)
                with c2.Else():
                    with tc.If(pid < 3) as c4:
                        emit(2)
                    with c4.Else():
                        emit(3)
            with c1.Else():
                with tc.If(pid < 6) as c5:
                    with tc.If(pid < 5) as c6:
                        emit(4)
                    with c6.Else():
                        emit(5)
                with c5.Else():
                    with tc.If(pid < 7) as c7:
                        emit(6)
                    with c7.Else():
                        emit(7)
    nc.compile()
    return nc


# ---------------------------------------------------------------- entry point
LAST_RESULT = {}


def kernel(q, k, v, cu_seqlens_q=None, cu_seqlens_kv=None,
           max_seqlen_q=None, max_seqlen_kv=None, batch_size=1,
           _trace=False, _trace_cores=None, **_):
    from concourse.bass_utils import run_bass_kernel_spmd

    q = np.asarray(q, dtype=np.float32)
    k = np.asarray(k, dtype=np.float32)
    v = np.asarray(v, dtype=np.float32)

    reorg, restore = _reorg_restore()
    mask, attn = _inspector(q, k)                      # [16, 60, 60]
    expop = _register_expbits_op()

    qr, kr, vr = q[reorg], k[reorg], v[reorg]          # [L, 16, 64]

    scheds_by_core = []
    in_maps = []
    for c in range(NCORES):
        heads = [HPC * c + h for h in range(HPC)]
        scheds_by_core.append(
            [_schedule(mask[h], attn[h]) for h in heads])
        qT = np.empty((HPC, 65, L), MMDT)
        kT = np.empty((HPC, 65, L), MMDT)
        for i, h in enumerate(heads):
            qT[i, :64] = qr[:, h, :].T * QSCALE
            qT[i, 64] = CROW_Q
            kT[i, :64] = kr[:, h, :].T
            kT[i, 64] = CROW_K
        vaug = np.empty((HPC, S, BLK, 65), MMDT)
        for i, h in enumerate(heads):
            vaug[i, :, :, :64] = vr[:, h, :].reshape(S, BLK, D)
            vaug[i, :, :, 64] = 1.0
        # SBUF-layout pack: [head, partition(token-in-block), block*65]
        vaug = np.ascontiguousarray(
            vaug.transpose(0, 2, 1, 3)).reshape(HPC, BLK, S * 65)
        in_maps.append({"qT": qT, "kT": kT, "vaug": vaug})

    nc = _build_program(scheds_by_core, expop)
    res = run_bass_kernel_spmd(nc, in_maps, list(range(NCORES)),
                               trace=_trace, trace_cores=_trace_cores)
    LAST_RESULT["exec_time_ns"] = res.exec_time_ns
    LAST_RESULT["mean_exec_time_ns"] = res.mean_exec_time_ns
    LAST_RESULT["res"] = res

    x_r = np.empty((L, NH, D), np.float32)
    for c in range(NCORES):
        out = res.results[c]["out"]        # [HPC, NWIN, 128, PVPACK*65] f16
        for h in range(HPC):
            _, zero_rows = scheds_by_core[c][h]
            til = out[h].astype(np.float32)            # [NWIN, 128, 390]
            til = til.reshape(NWIN, BLK, PVPACK, 65)
            num = til[:, :, :, :64]                    # [NWIN, 128, 6, 64]
            den = np.maximum(til[:, :, :, 64:65], 1e-30)
            xh = (num / den).transpose(0, 2, 1, 3)     # [NWIN, 6, 128, 64]
            xh = np.ascontiguousarray(xh).reshape(S, BLK, D)
            for qb in zero_rows:
                xh[qb] = 0.0
            x_r[:, HPC * c + h, :] = xh.reshape(L, D)
    x = x_r[restore]
    return x.reshape(int(batch_size), L, NH, D)


# revision 18
# speedup vs baseline: 1.8347x; 1.8347x over previous
"""Draft (block-sparse) attention kernel for Trainium2, 8 NeuronCores.

Strategy (v2)
-------------
* Head-parallel sharding: 16 heads -> 8 cores x 2 heads (361 kept blocks
  per head -> 722 (qb, kb) block pairs per core).
* Inspector / executor split: the draft map (pooled 60x60 attention +
  top-10% percentile mask) is a bitwise replica of the reference's jax
  ops on XLA-CPU; the block schedule is baked into the Bass program.
* Executor dataflow per pair:
      S'[kb_tok, qb_tok] = (K_kb)(Q'_qb)^T     PE fp16, K=64 contraction
                                               (no zero padding needed -
                                               matmul cost is column
                                               count, independent of K)
      P = 2^(S')                               Q' is pre-scaled on host
                                               by log2(e)/8, so softmax's
                                               exp(s/8) becomes 2^s'.
      acc[qb] += P^T @ [V_kb | 1]              PE fp16 -> PSUM fp32; the
                                               ones column accumulates
                                               the softmax denominator.
  The PSUM accumulators ship to HBM as fp16 (numerator + denominator);
  the host does the final divide, restore permutation and zero rows.
* exp is the serial bottleneck on one engine (ScalarE ~86us wall-to-wall
  in v1), so it is SPLIT between ScalarE (ACT Exp, exact) and VectorE
  (DVE): chunks of 12 pairs alternate between the engines by a weighted
  round-robin.  GPSIMD/Pool cannot read PSUM (BIR verifier) so it only
  issues DMA and memsets.
* DVE exp method (DVE_EXP):
    'pow'   - tensor_tensor(out, base=2.0 tile, s', AluOpType.pow)
    'trick' - int16 Schraudolph: y=s'*1024+B truncated to int16 IS the
              fp16 bit pattern of ~2^s' (max rel err ~3%).  To protect
              accuracy, chunks are weight-routed: within each 6-query-row
              window, pairs are sorted by draft-map weight relative to
              their row's max, ACT (exact) takes the heavy pairs, DVE the
              light tail.
* Output: per 6-row window a PSUM tile [128, 6*65] accumulates; after the
  window's last PV matmul it is copied (ACT/DVE alternating) to an SBUF
  staging tile as fp16 and DMA'd out immediately - no end-of-kernel DMA
  bulge.
* One tc.Switch (computed goto) dispatches the 8 per-core bodies instead
  of a binary If-tree (up to 3 far jumps -> 1).
"""

import math

import numpy as np

# ---------------------------------------------------------------- constants
L = 7680          # visual tokens (2 frames x 48 x 80)
NH = 16           # heads
D = 64            # head dim
S = 60            # pooled tokens = sparse blocks per side
BLK = 128         # tokens per block (L // S)
NCORES = 8
HPC = NH // NCORES  # heads per core
POOL_H, POOL_W, LATENT_H, LATENT_W = 8, 16, 48, 80
SPARSITY = 0.9

CHUNK_A = 8       # pairs per ScalarE exp chunk; A/V strictly alternate
CHUNK_V = 8       # pairs per DVE exp chunk
CHUNK = 8         # PSUM chunk tile size [128, 1024] fp32 = 2 banks
KROWS = 128       # q/k tile rows: 64 dims + const row + zero padding (fast
                  # LDWEIGHTS path; K=65 measured ~20% slower matmul pitch)
PVPACK = 7        # query rows per pv tile [128, 7*65] fp32 = 1820B (a matmul
                  # output slot must never straddle a 2KB PSUM bank boundary)
NWIN = (S + PVPACK - 1) // PVPACK  # windows per head (last is ragged: 4 rows)
MMDT = np.float16

FA = 0.55         # fraction of exp chunks on ScalarE (ACT); rest on DVE
DVE_EXP = "expbits"  # 'expbits' (custom DVE op, ~2e-3) or 'trick' (~3e-2)

# Scores are shipped so the PE emits y'' = 1024*log2(p_target) - 512:
# q is pre-scaled by 128*log2(e) (so s_raw*that = 1024*log2(e)/8*s_raw)
# and a 65th constant contraction row (q=-512, k=1) adds the -512.
QSCALE = 128.0 * math.log2(math.e)
CROW_Q = -512.0
CROW_K = 1.0

# EXPBITS custom-DVE op: magic-add floor-reduction + quadratic mantissa
# correction, B folded into the poly constant; the int16 output conversion
# + fp16 bitcast assemble the final float.  Max ripple ~2.1e-3; the mean
# +8.75-bit offset is matched on the ACT path via its activation bias so
# the two engines' outputs mix without a systematic scale step.
M2 = 1.5 * 2**33
EXPB_C0B = 15793.353872778089
EXPB_C1 = 1.0001873677319373
EXPB_C2 = 3.30059028135e-4
EXPB_K = 0.005906502815287413          # ln of the DVE path's mean scale
ACT_SCALE = math.log(2.0) / 1024.0     # exp(y''*scale + bias) == p_target
ACT_BIAS = 512.0 * math.log(2.0) / 1024.0 + EXPB_K
TRICK_B = 15360.0 + 512.0 - 44.0       # fallback: bits = y'' + TRICK_B


def _expbits_ref(in0, in1, s0, s1, imm2):
    ypp = in0.astype(np.float32)
    u = (ypp + np.float32(M2)).astype(np.float32)
    iF = (u - np.float32(M2)).astype(np.float32)
    f = (ypp - iF).astype(np.float32)
    t = ((np.float32(imm2) * f + np.float32(s1)) * f).astype(np.float32)
    t = (t + in1[:, :1].astype(np.float32)).astype(np.float32)
    t = (t + iF).astype(np.float32)
    return np.round(t).astype(np.float32)   # HW rounds on fp32->int16


def _register_expbits_op():
    import concourse.dve_ops as dvo
    from concourse.dve_spec import (Spec, Src0, C0, C1, C2, C3,
                                    _spill_c3_to_src1, lower)
    from concourse.dve_uop import DveOpSpec

    name = "EXPBITS_ANT"
    if name in dvo._SUB_OPCODE_FOR_NAME:
        return next(o for o in dvo.OPS if o.name == name)
    u = Src0 + C0
    iF = u - C0
    f = Src0 - iF
    body = ((C2 * f + C1) * f + C3) + iF
    spec = Spec(body=_spill_c3_to_src1(body), reference=_expbits_ref)
    row = dvo._CUSTOM_DVE_ROW_BASE + len(dvo.OPS)
    shas = {}
    for ver in ("v3", "v4"):
        tmp = DveOpSpec(name=name, opcode=row, uops=lower(spec, ver=ver),
                        rd1_en=True)
        shas[ver] = tmp.sha(ver)
    op = dvo.DveOp(name, spec, subdim=False, uops_sha=shas)
    dvo.OPS.append(op)
    dvo.CUSTOM_DVE_SPECS[name] = spec
    dvo._SUB_OPCODE_FOR_NAME[name] = row
    return op


def _reorg_restore():
    part = LATENT_W * POOL_H
    blk = LATENT_W
    sub = POOL_W
    bpp = part // blk
    spb = blk // sub
    pat = np.arange(part).reshape(bpp, spb, sub).transpose(1, 0, 2).reshape(-1)
    nparts = L // part
    reorg = (np.arange(nparts)[:, None] * part + pat[None, :]).reshape(-1)
    restore = np.argsort(reorg)
    return reorg, restore


def _inspector(qn: np.ndarray, kn: np.ndarray):
    """Replicate the reference draft-map + percentile mask bit-exactly on
    XLA-CPU.  Returns (mask, attn) - attn drives the exact-vs-approx exp
    routing."""
    import jax
    import jax.numpy as jnp

    with jax.default_device(jax.devices("cpu")[0]):
        q = jnp.asarray(qn)
        k = jnp.asarray(kn)
        nf = L // (LATENT_H * LATENT_W)

        def pool(x):
            x = x.reshape(nf, LATENT_H // POOL_H, POOL_H,
                          LATENT_W // POOL_W, POOL_W, NH, D)
            return x.mean(axis=(2, 4)).reshape(-1, NH, D)

        qs, ks = pool(q), pool(k)
        scores = jnp.einsum('lhd,mhd->hlm', qs, ks) / math.sqrt(D)
        attn = jax.nn.softmax(scores, axis=-1)
        n = S * S
        kk = int(SPARSITY * n)
        thr = jnp.sort(attn.reshape(NH, n), axis=-1)[:, kk - 1]
        mask = attn >= thr[:, None, None]
        return np.asarray(mask), np.asarray(attn)


def _schedule(mask_h: np.ndarray, attn_h: np.ndarray):
    """Per head: windows of PVPACK query rows.  Each window's pairs are
    sorted descending by draft weight relative to their row's max kept
    weight, so exact-exp chunks grab the heavy pairs first."""
    windows = []     # [ [(qb, kb), ...] sorted desc by relative weight ]
    zero_rows = []
    for qb in range(S):
        if not mask_h[qb].any():
            zero_rows.append(qb)
    for ti in range(NWIN):
        pairs = []
        for qb in range(ti * PVPACK, min(S, (ti + 1) * PVPACK)):
            kbs = np.nonzero(mask_h[qb])[0]
            if len(kbs) == 0:
                continue
            rmax = attn_h[qb, kbs].max()
            for kb in kbs:
                pairs.append((qb, int(kb), float(attn_h[qb, kb] / rmax)))
        pairs.sort(key=lambda p: -p[2])
        windows.append([(qb, kb) for qb, kb, _ in pairs])
    return windows, zero_rows


# ---------------------------------------------------------------- builder
def _emit_loads(nc, pools, dram):
    """Input loads, identical instructions on every core.  Ordered by first
    use: kT0/qT0 gate head-0 compute start (kT fully - every kb block is
    live immediately; qT in ascending query order), vaug0 next, then all
    of head 1."""
    import concourse.mybir as mybir

    f16 = mybir.dt.float16
    f32 = mybir.dt.float32
    qT_ap, kT_ap, vaug_ap = dram[0], dram[1], dram[2]

    qT = [pools["io"].tile([KROWS, L], f16, tag=f"qT{h}", name=f"qT{h}")
          for h in range(HPC)]
    kT = [pools["io"].tile([KROWS, L], f16, tag=f"kT{h}", name=f"kT{h}")
          for h in range(HPC)]
    vaug = [pools["io"].tile([128, S * 65], f16, tag=f"vaug{h}", name=f"vg{h}")
            for h in range(HPC)]
    c0b = pools["io"].tile([128, 1], f32, tag="c0b", name="c0b")
    nc.gpsimd.memset(c0b[:, :], EXPB_C0B)
    abias = pools["io"].tile([128, 1], f32, tag="abias", name="abias")
    nc.gpsimd.memset(abias[:, :], ACT_BIAS)

    third = L // 3
    vh = S * 65 // 2
    # head-0 kT split across all three queues (gates compute start), then
    # qT0 in query order, vaug0, then head 1
    nc.sync.dma_start(kT[0][:, 0:third], kT_ap[0][:, 0:third])
    nc.sync.dma_start(qT[0][:, 0:third], qT_ap[0][:, 0:third])
    nc.sync.dma_start(vaug[0][:, 0:vh], vaug_ap[0][:, 0:vh])
    nc.sync.dma_start(kT[1][:, 0:L // 2], kT_ap[1][:, 0:L // 2])
    nc.sync.dma_start(qT[1][:, 0:L // 2], qT_ap[1][:, 0:L // 2])
    nc.scalar.dma_start(kT[0][:, third:2 * third], kT_ap[0][:, third:2 * third])
    nc.scalar.dma_start(qT[0][:, third:2 * third], qT_ap[0][:, third:2 * third])
    nc.scalar.dma_start(vaug[1][:, 0:vh], vaug_ap[1][:, 0:vh])
    nc.gpsimd.dma_start(kT[0][:, 2 * third:L], kT_ap[0][:, 2 * third:L])
    nc.gpsimd.dma_start(qT[0][:, 2 * third:L], qT_ap[0][:, 2 * third:L])
    nc.gpsimd.dma_start(vaug[0][:, vh:], vaug_ap[0][:, vh:])
    nc.gpsimd.dma_start(kT[1][:, L // 2:], kT_ap[1][:, L // 2:])
    nc.gpsimd.dma_start(qT[1][:, L // 2:], qT_ap[1][:, L // 2:])
    nc.gpsimd.dma_start(vaug[1][:, vh:], vaug_ap[1][:, vh:])
    return qT, kT, vaug, c0b, abias


def _emit_core_compute(nc, tc, pools, tiles, dram, core, scheds, expop):
    import concourse.mybir as mybir

    f32 = mybir.dt.float32
    f16 = mybir.dt.float16
    i16 = mybir.dt.int16
    qT, kT, vaug, c0b, abias = tiles
    out_ap = dram[3]

    turn = ["A"]      # strict A/V alternation, carried across windows
    win_i = 0
    for h in range(HPC):
        windows, _ = scheds[h]
        for ti in range(NWIN):
            wpairs = windows[ti]
            if not wpairs:
                win_i += 1
                continue
            n = len(wpairs)
            # partition the weight-sorted pairs: A chunks (exact exp, heavy
            # pairs from the front), V chunks (approx, light tail), strictly
            # alternating with sizes matched to engine rates
            engs, plists = [], []
            front, back = 0, n
            while front < back:
                if turn[0] == "A":
                    sz = min(CHUNK_A, back - front)
                    plists.append(wpairs[front:front + sz])
                    front += sz
                    engs.append("A")
                    turn[0] = "V"
                else:
                    sz = min(CHUNK_V, back - front)
                    plists.append(wpairs[back - sz:back])
                    back -= sz
                    engs.append("V")
                    turn[0] = "A"

            pv = pools["pv"].tile([128, PVPACK * 65], f32, tag="pv",
                                  name=f"pv{core}_{win_i}")
            # stage 1: all chunks' S matmuls + exp (A/V alternating)
            pcs = []
            for e, plist in zip(engs, plists):
                cw = len(plist)
                sc = pools["schunk"].tile([128, CHUNK * BLK], f32,
                                          tag="schunk",
                                          name=f"sc{core}_{win_i}_{len(pcs)}")
                for i, (qb, kb) in enumerate(plist):
                    nc.tensor.matmul(
                        sc[:, i * BLK:(i + 1) * BLK],
                        lhsT=kT[h][:, kb * BLK:(kb + 1) * BLK],
                        rhs=qT[h][:, qb * BLK:(qb + 1) * BLK],
                        start=True, stop=True,
                    )
                nn = cw * BLK
                pc = pools["pchunk"].tile([128, CHUNK * BLK], f16,
                                          tag="pchunk",
                                          name=f"pc{core}_{win_i}_{len(pcs)}")
                if e == "A":
                    nc.scalar.activation(
                        pc[:, :nn], sc[:, :nn],
                        mybir.ActivationFunctionType.Exp,
                        scale=ACT_SCALE, bias=abias[:, :],
                    )
                elif DVE_EXP == "expbits":
                    nc.vector._custom_dve(
                        expop, out=pc[:, :nn].bitcast(i16), in0=sc[:, :nn],
                        in1=c0b[:, :], s0=M2, s1=EXPB_C1, imm2=EXPB_C2,
                    )
                else:
                    nc.vector.tensor_scalar(
                        pc[:, :nn].bitcast(i16), sc[:, :nn],
                        1.0, TRICK_B,
                        mybir.AluOpType.mult, mybir.AluOpType.add,
                    )
                pcs.append(pc)

            # stage 2: PV accumulation, each row's matmuls CONSECUTIVE so
            # PSUM start/stop groups never interleave
            row_pairs = {}
            for ci, plist in enumerate(plists):
                for i, (qb, kb) in enumerate(plist):
                    row_pairs.setdefault(qb, []).append((ci, i, kb))
            for qb in sorted(row_pairs):
                rp = row_pairs[qb]
                r = qb - ti * PVPACK
                for j, (ci, i, kb) in enumerate(rp):
                    nc.tensor.matmul(
                        pv[:, r * 65:r * 65 + 65],
                        lhsT=pcs[ci][:, i * BLK:(i + 1) * BLK],
                        rhs=vaug[h][:, kb * 65:(kb + 1) * 65],
                        start=(j == 0), stop=(j == len(rp) - 1),
                        skip_group_check=True,
                    )

            st = pools["stage"].tile([128, PVPACK * 65], f16, tag="stage",
                                     name=f"st{core}_{win_i}")
            nc.vector.tensor_copy(st[:, :], pv[:, :])
            q = nc.sync if (win_i % 2 == 0) else nc.gpsimd
            q.dma_start(out_ap[h][ti], st[:, :])
            win_i += 1


def _build_program(scheds_by_core, expop):
    from contextlib import ExitStack

    import concourse.mybir as mybir
    import concourse.tile as tile
    from concourse import bacc

    f16 = mybir.dt.float16
    nc = bacc.Bacc("TRN2", target_bir_lowering=False, debug=False,
                   num_devices=NCORES)
    qT_ap = nc.dram_tensor("qT", [HPC, KROWS, L], f16,
                           kind="ExternalInput").ap()
    kT_ap = nc.dram_tensor("kT", [HPC, KROWS, L], f16,
                           kind="ExternalInput").ap()
    vaug_ap = nc.dram_tensor("vaug", [HPC, BLK, S * 65], f16,
                             kind="ExternalInput").ap()
    out_ap = nc.dram_tensor("out", [HPC, NWIN, BLK, PVPACK * 65], f16,
                            kind="ExternalOutput").ap()

    dram = (qT_ap, kT_ap, vaug_ap, out_ap)

    with tile.TileContext(nc) as tc:
        with ExitStack() as ctx:
            pools = {
                "io": ctx.enter_context(tc.tile_pool(name="io", bufs=1)),
                "schunk": ctx.enter_context(
                    tc.tile_pool(name="schunk", bufs=3, space="PSUM")),
                "pchunk": ctx.enter_context(
                    tc.tile_pool(name="pchunk", bufs=16)),
                "pv": ctx.enter_context(
                    tc.tile_pool(name="pv", bufs=2, space="PSUM")),
                "stage": ctx.enter_context(tc.tile_pool(name="stage", bufs=4)),
            }
            pid = nc.partition_id()
            tiles = _emit_loads(nc, pools, dram)
            for core in tc.Switch(pid, NCORES):
                _emit_core_compute(nc, tc, pools, tiles, dram, core,
                                   scheds_by_core[core], expop)
    nc.compile()
    return nc


# ---------------------------------------------------------------- entry point
LAST_RESULT = {}


def kernel(q, k, v, cu_seqlens_q=None, cu_seqlens_kv=None,
           max_seqlen_q=None, max_seqlen_kv=None, batch_size=1,
           _trace=False, _trace_cores=None, **_):
    from concourse.bass_utils import run_bass_kernel_spmd

    q = np.asarray(q, dtype=np.float32)
    k = np.asarray(k, dtype=np.float32)
    v = np.asarray(v, dtype=np.float32)

    reorg, restore = _reorg_restore()
    mask, attn = _inspector(q, k)                      # [16, 60, 60]
    expop = _register_expbits_op()

    qr, kr, vr = q[reorg], k[reorg], v[reorg]          # [L, 16, 64]

    scheds_by_core = []
    in_maps = []
    for c in range(NCORES):
        heads = [HPC * c + h for h in range(HPC)]
        scheds_by_core.append(
            [_schedule(mask[h], attn[h]) for h in heads])
        qT = np.zeros((HPC, KROWS, L), MMDT)
        kT = np.zeros((HPC, KROWS, L), MMDT)
        for i, h in enumerate(heads):
            qT[i, :64] = qr[:, h, :].T * QSCALE
            qT[i, 64] = CROW_Q
            kT[i, :64] = kr[:, h, :].T
            kT[i, 64] = CROW_K
        vaug = np.empty((HPC, S, BLK, 65), MMDT)
        for i, h in enumerate(heads):
            vaug[i, :, :, :64] = vr[:, h, :].reshape(S, BLK, D)
            vaug[i, :, :, 64] = 1.0
        # SBUF-layout pack: [head, partition(token-in-block), block*65]
        vaug = np.ascontiguousarray(
            vaug.transpose(0, 2, 1, 3)).reshape(HPC, BLK, S * 65)
        in_maps.append({"qT": qT, "kT": kT, "vaug": vaug})

    nc = _build_program(scheds_by_core, expop)
    res = run_bass_kernel_spmd(nc, in_maps, list(range(NCORES)),
                               trace=_trace, trace_cores=_trace_cores)
    LAST_RESULT["exec_time_ns"] = res.exec_time_ns
    LAST_RESULT["mean_exec_time_ns"] = res.mean_exec_time_ns
    LAST_RESULT["res"] = res

    x_r = np.empty((L, NH, D), np.float32)
    for c in range(NCORES):
        out = res.results[c]["out"]        # [HPC, NWIN, 128, PVPACK*65] f16
        for h in range(HPC):
            _, zero_rows = scheds_by_core[c][h]
            til = out[h].astype(np.float32)            # [NWIN, 128, PVPACK*65]
            til = til.reshape(NWIN, BLK, PVPACK, 65)
            num = til[:, :, :, :64]
            den = np.maximum(til[:, :, :, 64:65], 1e-30)
            xh = (num / den).transpose(0, 2, 1, 3)     # [NWIN, PVPACK, 128, 64]
            xh = np.ascontiguousarray(xh).reshape(NWIN * PVPACK, BLK, D)[:S]
            for qb in zero_rows:
                xh[qb] = 0.0
            x_r[:, HPC * c + h, :] = xh.reshape(L, D)
    x = x_r[restore]
    return x.reshape(int(batch_size), L, NH, D)


# revision 19
# speedup vs baseline: 1.9081x; 1.0400x over previous
"""Draft (block-sparse) attention kernel for Trainium2, 8 NeuronCores.

Strategy (v2)
-------------
* Head-parallel sharding: 16 heads -> 8 cores x 2 heads (361 kept blocks
  per head -> 722 (qb, kb) block pairs per core).
* Inspector / executor split: the draft map (pooled 60x60 attention +
  top-10% percentile mask) is a bitwise replica of the reference's jax
  ops on XLA-CPU; the block schedule is baked into the Bass program.
* Executor dataflow per pair:
      S'[kb_tok, qb_tok] = (K_kb)(Q'_qb)^T     PE fp16, K=64 contraction
                                               (no zero padding needed -
                                               matmul cost is column
                                               count, independent of K)
      P = 2^(S')                               Q' is pre-scaled on host
                                               by log2(e)/8, so softmax's
                                               exp(s/8) becomes 2^s'.
      acc[qb] += P^T @ [V_kb | 1]              PE fp16 -> PSUM fp32; the
                                               ones column accumulates
                                               the softmax denominator.
  The PSUM accumulators ship to HBM as fp16 (numerator + denominator);
  the host does the final divide, restore permutation and zero rows.
* exp is the serial bottleneck on one engine (ScalarE ~86us wall-to-wall
  in v1), so it is SPLIT between ScalarE (ACT Exp, exact) and VectorE
  (DVE): chunks of 12 pairs alternate between the engines by a weighted
  round-robin.  GPSIMD/Pool cannot read PSUM (BIR verifier) so it only
  issues DMA and memsets.
* DVE exp method (DVE_EXP):
    'pow'   - tensor_tensor(out, base=2.0 tile, s', AluOpType.pow)
    'trick' - int16 Schraudolph: y=s'*1024+B truncated to int16 IS the
              fp16 bit pattern of ~2^s' (max rel err ~3%).  To protect
              accuracy, chunks are weight-routed: within each 6-query-row
              window, pairs are sorted by draft-map weight relative to
              their row's max, ACT (exact) takes the heavy pairs, DVE the
              light tail.
* Output: per 6-row window a PSUM tile [128, 6*65] accumulates; after the
  window's last PV matmul it is copied (ACT/DVE alternating) to an SBUF
  staging tile as fp16 and DMA'd out immediately - no end-of-kernel DMA
  bulge.
* One tc.Switch (computed goto) dispatches the 8 per-core bodies instead
  of a binary If-tree (up to 3 far jumps -> 1).
"""

import math

import numpy as np

# ---------------------------------------------------------------- constants
L = 7680          # visual tokens (2 frames x 48 x 80)
NH = 16           # heads
D = 64            # head dim
S = 60            # pooled tokens = sparse blocks per side
BLK = 128         # tokens per block (L // S)
NCORES = 8
HPC = NH // NCORES  # heads per core
POOL_H, POOL_W, LATENT_H, LATENT_W = 8, 16, 48, 80
SPARSITY = 0.9

CHUNK_A = 8       # pairs per ScalarE exp chunk; A/V strictly alternate
CHUNK_V = 8       # pairs per DVE exp chunk
CHUNK = 8         # PSUM chunk tile size [128, 1024] fp32 = 2 banks
KROWS = 128       # q/k tile rows: 64 dims + const row + zero padding (fast
                  # LDWEIGHTS path; K=65 measured ~20% slower matmul pitch)
PVPACK = 7        # query rows per pv tile [128, 7*65] fp32 = 1820B (a matmul
                  # output slot must never straddle a 2KB PSUM bank boundary)
NWIN = (S + PVPACK - 1) // PVPACK  # windows per head (last is ragged: 4 rows)
MMDT = np.float16

FA = 0.55         # fraction of exp chunks on ScalarE (ACT); rest on DVE
DVE_EXP = "expbits"  # 'expbits' (custom DVE op, ~2e-3) or 'trick' (~3e-2)

# Scores are shipped so the PE emits y'' = 1024*log2(p_target) - 512:
# q is pre-scaled by 128*log2(e) (so s_raw*that = 1024*log2(e)/8*s_raw)
# and a 65th constant contraction row (q=-512, k=1) adds the -512.
QSCALE = 128.0 * math.log2(math.e)
CROW_Q = -512.0
CROW_K = 1.0

# EXPBITS custom-DVE op: magic-add floor-reduction + quadratic mantissa
# correction, B folded into the poly constant; the int16 output conversion
# + fp16 bitcast assemble the final float.  Max ripple ~2.1e-3; the mean
# +8.75-bit offset is matched on the ACT path via its activation bias so
# the two engines' outputs mix without a systematic scale step.
M2 = 1.5 * 2**33
EXPB_C0B = 15793.353872778089
EXPB_C1 = 1.0001873677319373
EXPB_C2 = 3.30059028135e-4
EXPB_K = 0.005906502815287413          # ln of the DVE path's mean scale
ACT_SCALE = math.log(2.0) / 1024.0     # exp(y''*scale + bias) == p_target
ACT_BIAS = 512.0 * math.log(2.0) / 1024.0 + EXPB_K
TRICK_B = 15360.0 + 512.0 - 44.0       # fallback: bits = y'' + TRICK_B


def _expbits_ref(in0, in1, s0, s1, imm2):
    ypp = in0.astype(np.float32)
    u = (ypp + np.float32(M2)).astype(np.float32)
    iF = (u - np.float32(M2)).astype(np.float32)
    f = (ypp - iF).astype(np.float32)
    t = ((np.float32(imm2) * f + np.float32(s1)) * f).astype(np.float32)
    t = (t + in1[:, :1].astype(np.float32)).astype(np.float32)
    t = (t + iF).astype(np.float32)
    return np.round(t).astype(np.float32)   # HW rounds on fp32->int16


def _register_expbits_op():
    import concourse.dve_ops as dvo
    from concourse.dve_spec import (Spec, Src0, C0, C1, C2, C3,
                                    _spill_c3_to_src1, lower)
    from concourse.dve_uop import DveOpSpec

    name = "EXPBITS_ANT"
    if name in dvo._SUB_OPCODE_FOR_NAME:
        return next(o for o in dvo.OPS if o.name == name)
    u = Src0 + C0
    iF = u - C0
    f = Src0 - iF
    body = ((C2 * f + C1) * f + C3) + iF
    spec = Spec(body=_spill_c3_to_src1(body), reference=_expbits_ref)
    row = dvo._CUSTOM_DVE_ROW_BASE + len(dvo.OPS)
    shas = {}
    for ver in ("v3", "v4"):
        tmp = DveOpSpec(name=name, opcode=row, uops=lower(spec, ver=ver),
                        rd1_en=True)
        shas[ver] = tmp.sha(ver)
    op = dvo.DveOp(name, spec, subdim=False, uops_sha=shas)
    dvo.OPS.append(op)
    dvo.CUSTOM_DVE_SPECS[name] = spec
    dvo._SUB_OPCODE_FOR_NAME[name] = row
    return op


def _reorg_restore():
    part = LATENT_W * POOL_H
    blk = LATENT_W
    sub = POOL_W
    bpp = part // blk
    spb = blk // sub
    pat = np.arange(part).reshape(bpp, spb, sub).transpose(1, 0, 2).reshape(-1)
    nparts = L // part
    reorg = (np.arange(nparts)[:, None] * part + pat[None, :]).reshape(-1)
    restore = np.argsort(reorg)
    return reorg, restore


def _inspector(qn: np.ndarray, kn: np.ndarray):
    """Replicate the reference draft-map + percentile mask bit-exactly on
    XLA-CPU.  Returns (mask, attn) - attn drives the exact-vs-approx exp
    routing."""
    import jax
    import jax.numpy as jnp

    with jax.default_device(jax.devices("cpu")[0]):
        q = jnp.asarray(qn)
        k = jnp.asarray(kn)
        nf = L // (LATENT_H * LATENT_W)

        def pool(x):
            x = x.reshape(nf, LATENT_H // POOL_H, POOL_H,
                          LATENT_W // POOL_W, POOL_W, NH, D)
            return x.mean(axis=(2, 4)).reshape(-1, NH, D)

        qs, ks = pool(q), pool(k)
        scores = jnp.einsum('lhd,mhd->hlm', qs, ks) / math.sqrt(D)
        attn = jax.nn.softmax(scores, axis=-1)
        n = S * S
        kk = int(SPARSITY * n)
        thr = jnp.sort(attn.reshape(NH, n), axis=-1)[:, kk - 1]
        mask = attn >= thr[:, None, None]
        return np.asarray(mask), np.asarray(attn)


def _schedule(mask_h: np.ndarray, attn_h: np.ndarray):
    """Per head: windows of PVPACK query rows.  Each window's pairs are
    sorted descending by draft weight relative to their row's max kept
    weight, so exact-exp chunks grab the heavy pairs first."""
    windows = []     # [ [(qb, kb), ...] sorted desc by relative weight ]
    zero_rows = []
    for qb in range(S):
        if not mask_h[qb].any():
            zero_rows.append(qb)
    for ti in range(NWIN):
        pairs = []
        for qb in range(ti * PVPACK, min(S, (ti + 1) * PVPACK)):
            kbs = np.nonzero(mask_h[qb])[0]
            if len(kbs) == 0:
                continue
            rmax = attn_h[qb, kbs].max()
            for kb in kbs:
                pairs.append((qb, int(kb), float(attn_h[qb, kb] / rmax)))
        pairs.sort(key=lambda p: -p[2])
        windows.append([(qb, kb) for qb, kb, _ in pairs])
    return windows, zero_rows


# ---------------------------------------------------------------- builder
def _emit_loads(nc, pools, dram):
    """Input loads, identical instructions on every core.  Ordered by first
    use: kT0/qT0 gate head-0 compute start (kT fully - every kb block is
    live immediately; qT in ascending query order), vaug0 next, then all
    of head 1."""
    import concourse.mybir as mybir

    f16 = mybir.dt.float16
    f32 = mybir.dt.float32
    qT_ap, kT_ap, vaug_ap = dram[0], dram[1], dram[2]

    qT = [pools["io"].tile([KROWS, L], f16, tag=f"qT{h}", name=f"qT{h}")
          for h in range(HPC)]
    kT = [pools["io"].tile([KROWS, L], f16, tag=f"kT{h}", name=f"kT{h}")
          for h in range(HPC)]
    vaug = [pools["io"].tile([128, S * 65], f16, tag=f"vaug{h}", name=f"vg{h}")
            for h in range(HPC)]
    c0b = pools["io"].tile([128, 1], f32, tag="c0b", name="c0b")
    nc.gpsimd.memset(c0b[:, :], EXPB_C0B)
    abias = pools["io"].tile([128, 1], f32, tag="abias", name="abias")
    nc.gpsimd.memset(abias[:, :], ACT_BIAS)

    third = L // 3
    q8 = L // 6    # 1280-col qT slices, landed just-in-time in query order
    vh = S * 65 // 2
    # kT0 split three ways gates compute start; vaug0 + early qT0 next
    nc.sync.dma_start(kT[0][:, 0:third], kT_ap[0][:, 0:third])
    nc.sync.dma_start(qT[0][:, 0:q8], qT_ap[0][:, 0:q8])
    nc.sync.dma_start(vaug[0][:, 0:vh], vaug_ap[0][:, 0:vh])
    nc.sync.dma_start(qT[0][:, 2 * q8:3 * q8], qT_ap[0][:, 2 * q8:3 * q8])
    nc.sync.dma_start(kT[1][:, 0:L // 2], kT_ap[1][:, 0:L // 2])
    nc.sync.dma_start(qT[1][:, 0:L // 2], qT_ap[1][:, 0:L // 2])
    nc.scalar.dma_start(kT[0][:, third:2 * third], kT_ap[0][:, third:2 * third])
    nc.scalar.dma_start(vaug[0][:, vh:], vaug_ap[0][:, vh:])
    nc.scalar.dma_start(qT[0][:, 3 * q8:4 * q8], qT_ap[0][:, 3 * q8:4 * q8])
    nc.scalar.dma_start(vaug[1][:, 0:vh], vaug_ap[1][:, 0:vh])
    nc.gpsimd.dma_start(kT[0][:, 2 * third:L], kT_ap[0][:, 2 * third:L])
    nc.gpsimd.dma_start(qT[0][:, q8:2 * q8], qT_ap[0][:, q8:2 * q8])
    nc.gpsimd.dma_start(qT[0][:, 4 * q8:L], qT_ap[0][:, 4 * q8:L])
    nc.gpsimd.dma_start(kT[1][:, L // 2:], kT_ap[1][:, L // 2:])
    nc.gpsimd.dma_start(qT[1][:, L // 2:], qT_ap[1][:, L // 2:])
    nc.gpsimd.dma_start(vaug[1][:, vh:], vaug_ap[1][:, vh:])
    return qT, kT, vaug, c0b, abias


def _emit_core_compute(nc, tc, pools, tiles, dram, core, scheds, expop):
    import concourse.mybir as mybir

    f32 = mybir.dt.float32
    f16 = mybir.dt.float16
    i16 = mybir.dt.int16
    qT, kT, vaug, c0b, abias = tiles
    out_ap = dram[3]

    turn = ["A"]      # strict A/V alternation, carried across windows
    win_i = 0
    for h in range(HPC):
        windows, _ = scheds[h]
        for ti in range(NWIN):
            wpairs = windows[ti]
            if not wpairs:
                win_i += 1
                continue
            n = len(wpairs)
            # partition the weight-sorted pairs: A chunks (exact exp, heavy
            # pairs from the front), V chunks (approx, light tail), strictly
            # alternating with sizes matched to engine rates
            engs, plists = [], []
            front, back = 0, n
            while front < back:
                if turn[0] == "A":
                    sz = min(CHUNK_A, back - front)
                    plists.append(wpairs[front:front + sz])
                    front += sz
                    engs.append("A")
                    turn[0] = "V"
                else:
                    sz = min(CHUNK_V, back - front)
                    plists.append(wpairs[back - sz:back])
                    back -= sz
                    engs.append("V")
                    turn[0] = "A"

            pv = pools["pv"].tile([128, PVPACK * 65], f32, tag="pv",
                                  name=f"pv{core}_{win_i}")
            # stage 1: all chunks' S matmuls + exp (A/V alternating)
            pcs = []
            for e, plist in zip(engs, plists):
                cw = len(plist)
                sc = pools["schunk"].tile([128, CHUNK * BLK], f32,
                                          tag="schunk",
                                          name=f"sc{core}_{win_i}_{len(pcs)}")
                for i, (qb, kb) in enumerate(plist):
                    nc.tensor.matmul(
                        sc[:, i * BLK:(i + 1) * BLK],
                        lhsT=kT[h][:, kb * BLK:(kb + 1) * BLK],
                        rhs=qT[h][:, qb * BLK:(qb + 1) * BLK],
                        start=True, stop=True,
                    )
                nn = cw * BLK
                pc = pools["pchunk"].tile([128, CHUNK * BLK], f16,
                                          tag="pchunk",
                                          name=f"pc{core}_{win_i}_{len(pcs)}")
                if e == "A":
                    nc.scalar.activation(
                        pc[:, :nn], sc[:, :nn],
                        mybir.ActivationFunctionType.Exp,
                        scale=ACT_SCALE, bias=abias[:, :],
                    )
                elif DVE_EXP == "expbits":
                    nc.vector._custom_dve(
                        expop, out=pc[:, :nn].bitcast(i16), in0=sc[:, :nn],
                        in1=c0b[:, :], s0=M2, s1=EXPB_C1, imm2=EXPB_C2,
                    )
                else:
                    nc.vector.tensor_scalar(
                        pc[:, :nn].bitcast(i16), sc[:, :nn],
                        1.0, TRICK_B,
                        mybir.AluOpType.mult, mybir.AluOpType.add,
                    )
                pcs.append(pc)

            # stage 2: PV accumulation, each row's matmuls CONSECUTIVE so
            # PSUM start/stop groups never interleave
            row_pairs = {}
            for ci, plist in enumerate(plists):
                for i, (qb, kb) in enumerate(plist):
                    row_pairs.setdefault(qb, []).append((ci, i, kb))
            for qb in sorted(row_pairs):
                rp = row_pairs[qb]
                r = qb - ti * PVPACK
                for j, (ci, i, kb) in enumerate(rp):
                    nc.tensor.matmul(
                        pv[:, r * 65:r * 65 + 65],
                        lhsT=pcs[ci][:, i * BLK:(i + 1) * BLK],
                        rhs=vaug[h][:, kb * 65:(kb + 1) * 65],
                        start=(j == 0), stop=(j == len(rp) - 1),
                        skip_group_check=True,
                    )

            st = pools["stage"].tile([128, PVPACK * 65], f16, tag="stage",
                                     name=f"st{core}_{win_i}")
            nc.vector.tensor_copy(st[:, :], pv[:, :])
            q = nc.sync if (win_i % 2 == 0) else nc.gpsimd
            q.dma_start(out_ap[h][ti], st[:, :])
            win_i += 1


def _build_program(scheds_by_core, expop):
    from contextlib import ExitStack

    import concourse.mybir as mybir
    import concourse.tile as tile
    from concourse import bacc

    f16 = mybir.dt.float16
    nc = bacc.Bacc("TRN2", target_bir_lowering=False, debug=False,
                   num_devices=NCORES)
    qT_ap = nc.dram_tensor("qT", [HPC, KROWS, L], f16,
                           kind="ExternalInput").ap()
    kT_ap = nc.dram_tensor("kT", [HPC, KROWS, L], f16,
                           kind="ExternalInput").ap()
    vaug_ap = nc.dram_tensor("vaug", [HPC, BLK, S * 65], f16,
                             kind="ExternalInput").ap()
    out_ap = nc.dram_tensor("out", [HPC, NWIN, BLK, PVPACK * 65], f16,
                            kind="ExternalOutput").ap()

    dram = (qT_ap, kT_ap, vaug_ap, out_ap)

    with tile.TileContext(nc) as tc:
        with ExitStack() as ctx:
            pools = {
                "io": ctx.enter_context(tc.tile_pool(name="io", bufs=1)),
                "schunk": ctx.enter_context(
                    tc.tile_pool(name="schunk", bufs=3, space="PSUM")),
                "pchunk": ctx.enter_context(
                    tc.tile_pool(name="pchunk", bufs=16)),
                "pv": ctx.enter_context(
                    tc.tile_pool(name="pv", bufs=2, space="PSUM")),
                "stage": ctx.enter_context(tc.tile_pool(name="stage", bufs=4)),
            }
            pid = nc.partition_id()
            tiles = _emit_loads(nc, pools, dram)
            for core in tc.Switch(pid, NCORES):
                _emit_core_compute(nc, tc, pools, tiles, dram, core,
                                   scheds_by_core[core], expop)
    nc.compile()
    return nc


# ---------------------------------------------------------------- entry point
LAST_RESULT = {}


def kernel(q, k, v, cu_seqlens_q=None, cu_seqlens_kv=None,
           max_seqlen_q=None, max_seqlen_kv=None, batch_size=1,
           _trace=False, _trace_cores=None, **_):
    from concourse.bass_utils import run_bass_kernel_spmd

    q = np.asarray(q, dtype=np.float32)
    k = np.asarray(k, dtype=np.float32)
    v = np.asarray(v, dtype=np.float32)

    reorg, restore = _reorg_restore()
    mask, attn = _inspector(q, k)                      # [16, 60, 60]
    expop = _register_expbits_op()

    qr, kr, vr = q[reorg], k[reorg], v[reorg]          # [L, 16, 64]

    scheds_by_core = []
    in_maps = []
    for c in range(NCORES):
        heads = [HPC * c + h for h in range(HPC)]
        scheds_by_core.append(
            [_schedule(mask[h], attn[h]) for h in heads])
        qT = np.zeros((HPC, KROWS, L), MMDT)
        kT = np.zeros((HPC, KROWS, L), MMDT)
        for i, h in enumerate(heads):
            qT[i, :64] = qr[:, h, :].T * QSCALE
            qT[i, 64] = CROW_Q
            kT[i, :64] = kr[:, h, :].T
            kT[i, 64] = CROW_K
        vaug = np.empty((HPC, S, BLK, 65), MMDT)
        for i, h in enumerate(heads):
            vaug[i, :, :, :64] = vr[:, h, :].reshape(S, BLK, D)
            vaug[i, :, :, 64] = 1.0
        # SBUF-layout pack: [head, partition(token-in-block), block*65]
        vaug = np.ascontiguousarray(
            vaug.transpose(0, 2, 1, 3)).reshape(HPC, BLK, S * 65)
        in_maps.append({"qT": qT, "kT": kT, "vaug": vaug})

    nc = _build_program(scheds_by_core, expop)
    res = run_bass_kernel_spmd(nc, in_maps, list(range(NCORES)),
                               trace=_trace, trace_cores=_trace_cores)
    LAST_RESULT["exec_time_ns"] = res.exec_time_ns
    LAST_RESULT["mean_exec_time_ns"] = res.mean_exec_time_ns
    LAST_RESULT["res"] = res

    x_r = np.empty((L, NH, D), np.float32)
    for c in range(NCORES):
        out = res.results[c]["out"]        # [HPC, NWIN, 128, PVPACK*65] f16
        for h in range(HPC):
            _, zero_rows = scheds_by_core[c][h]
            til = out[h].astype(np.float32)            # [NWIN, 128, PVPACK*65]
            til = til.reshape(NWIN, BLK, PVPACK, 65)
            num = til[:, :, :, :64]
            den = np.maximum(til[:, :, :, 64:65], 1e-30)
            xh = (num / den).transpose(0, 2, 1, 3)     # [NWIN, PVPACK, 128, 64]
            xh = np.ascontiguousarray(xh).reshape(NWIN * PVPACK, BLK, D)[:S]
            for qb in zero_rows:
                xh[qb] = 0.0
            x_r[:, HPC * c + h, :] = xh.reshape(L, D)
    x = x_r[restore]
    return x.reshape(int(batch_size), L, NH, D)
